# revision 1
# baseline (speedup 1.0000x reference)
"""Trainium2 Bass kernel for CustomMultiHeadAttention (B=2, L=2048, D=512, H=8).

Sharding: 8 cores = 2 batches x 4 head-pairs. Each core computes, for its
batch b and its 2 heads, the partial output (O_h @ Wo_h summed over its
heads), transposed: poutT [512, 2048]. Host sums the 4 partials per batch,
transposes, and adds bo.

Device-side math per core (all masking folded into matmul contractions):
  Qh = (q[b]*qm) @ WqT_cols + qm*bq_cols          (masked q rows -> exactly 0)
  Kh = k[b] @ WkT_cols/8 + bk_cols/8
  E[k,q] = Kh.Qh + (kb[k]-c)*qm[q] + c            via 2 extra contraction rows
           (kb = -1e4 for masked keys, c = ln(1/2048))
    -> unmasked q: E = s + kb  (masked keys underflow to 0 in exp)
    -> masked q:   E = c       (exp = 1/2048 uniform; denom = 1)
  PT = exp(E)   [k, q] layout
  outT = [Vp | 1]^T @ PT   (Vp = coef * Vh; ones column yields denom row)
  O = outT[0:64] / outT[64]
  poutT[d, q] += Wo_h[:, d] . O[:, q]

Emission order keeps the PE dense (HAM warm): q-proj, k-proj, first
scores+exp unit, then v-proj/transposes, then the remaining attention units.
"""

import math
import os

os.environ.setdefault("MYCRO_LOCAL_CACHE", "1")

import numpy as np

import concourse.bass as bass
import concourse.tile as tile
from concourse import bacc
from concourse import mybir
from concourse.bass_utils import run_bass_kernel_spmd
from concourse.masks import make_identity

B = 2
L = 2048
DM = 512
H = 8
DH = 64
NCORES = 8
HPC = 2           # heads per core
DH2 = HPC * DH    # 128
NKT = L // 128    # 16 k tiles
QH = 1024         # q chunk for attention phase
NQH = L // QH     # 2
C_LN = -math.log(L)
NEG = -10000.0

F32 = mybir.dt.float32
F32R = mybir.dt.float32r
BF16 = mybir.dt.bfloat16

ATT_DT = BF16     # exp output (PT), V'
MM_DT = F32R      # PE-streamed f32 operands: full rate, ~tf32 rounding

TRACE = False
LAST_RESULT = None

AUX_QM, AUX_KBMC, AUX_ONES, AUX_CLN = 0, 1, 2, 3


def build_nc(with_bias: bool):
    nc = bacc.Bacc(None, target_bir_lowering=False)

    xqT_d = nc.declare_dram_parameter("xqT", [DM, L], MM_DT, isOutput=False)
    xkT_d = nc.declare_dram_parameter("xkT", [DM, L], MM_DT, isOutput=False)
    xvT_d = nc.declare_dram_parameter("xvT", [DM, L], MM_DT, isOutput=False)
    wqs_d = nc.declare_dram_parameter("wqs", [DM, DH2], MM_DT, isOutput=False)
    wks_d = nc.declare_dram_parameter("wks", [DM, DH2], MM_DT, isOutput=False)
    wvs_d = nc.declare_dram_parameter("wvs", [DM, DH2], MM_DT, isOutput=False)
    if with_bias:
        wbias_d = nc.declare_dram_parameter(
            "wbias", [1, 4 * DH2], MM_DT, isOutput=False
        )
    wos_d = nc.declare_dram_parameter("wos", [DH2, DM], MM_DT, isOutput=False)
    aux_d = nc.declare_dram_parameter("aux", [4, L], MM_DT, isOutput=False)
    coef_d = nc.declare_dram_parameter("coef", [1, L], F32, isOutput=False)
    pout_d = nc.declare_dram_parameter("poutT", [DM, L], F32, isOutput=True)

    with tile.TileContext(nc) as tc:
        with (
            tc.tile_pool(name="const", bufs=1) as const,
            tc.tile_pool(name="qek", bufs=1) as qek,
            tc.tile_pool(name="xin", bufs=4) as xin,
            tc.tile_pool(name="vtmp", bufs=1) as vtmp,
            tc.tile_pool(name="ptp", bufs=2) as ptp,
            tc.tile_pool(name="sbB", bufs=1) as sbB,
            tc.tile_pool(name="ps", bufs=1, space="PSUM") as ps,
        ):
            # ---- constants ----
            ident = const.tile([128, 128], F32)
            make_identity(nc, ident)
            wq_sb = const.tile([128, 4, DH2], MM_DT)
            nc.sync.dma_start(
                out=wq_sb, in_=wqs_d[:, :].rearrange("(t p) m -> p t m", p=128)
            )
            wk_sb = const.tile([128, 4, DH2], MM_DT)
            wv_sb = const.tile([128, 4, DH2], MM_DT)
            wo_sb = [const.tile([DH, DM], MM_DT, name=f"wo{h}") for h in range(HPC)]
            coef_sb = const.tile([128, NKT], F32)
            if with_bias:
                wb_sb = const.tile([1, 4 * DH2], MM_DT)
                nc.sync.dma_start(out=wb_sb, in_=wbias_d[:, :])
                qm_sb = const.tile([1, L], MM_DT)
                nc.sync.dma_start(out=qm_sb, in_=aux_d[AUX_QM : AUX_QM + 1, :])
                ones_sb = const.tile([1, L], MM_DT)
                nc.sync.dma_start(out=ones_sb, in_=aux_d[AUX_ONES : AUX_ONES + 1, :])

            # ---- persistent per-head operands ----
            QE = [qek.tile([66, L], MM_DT, name=f"QE{h}") for h in range(HPC)]
            KE = [qek.tile([66, L], MM_DT, name=f"KE{h}") for h in range(HPC)]
            Vp = [
                qek.tile([128, NKT, DH + 1], ATT_DT, name=f"Vp{h}") for h in range(HPC)
            ]
            # mask/bias rows of the extended operands (DMA direct from host aux)
            for h in range(HPC):
                nc.sync.dma_start(
                    out=QE[h][64:65, :], in_=aux_d[AUX_QM : AUX_QM + 1, :]
                )
                nc.sync.dma_start(
                    out=QE[h][65:66, :], in_=aux_d[AUX_ONES : AUX_ONES + 1, :]
                )
                nc.sync.dma_start(
                    out=KE[h][64:65, :], in_=aux_d[AUX_KBMC : AUX_KBMC + 1, :]
                )
                nc.sync.dma_start(
                    out=KE[h][65:66, :], in_=aux_d[AUX_CLN : AUX_CLN + 1, :]
                )
                nc.vector.memset(Vp[h][:, :, DH : DH + 1], 1.0)

            def emit_proj(pname, xdram, w_sb, brow, brhs, evict):
                xts = []
                for t in range(4):
                    xt = xin.tile([128, L], MM_DT, tag="xin", name=f"x{pname}{t}")
                    nc.sync.dma_start(out=xt, in_=xdram[t * 128 : (t + 1) * 128, :])
                    xts.append(xt)
                for ch in range(4):
                    sl = slice(ch * 512, (ch + 1) * 512)
                    psp = ps.tile([128, 512], F32, tag="small", bufs=2, name="psp")
                    for t in range(4):
                        nc.tensor.matmul(
                            psp,
                            lhsT=w_sb[:, t, :],
                            rhs=xts[t][:, sl],
                            start=(t == 0),
                            stop=(t == 3 and not with_bias),
                        )
                    if with_bias:
                        nc.tensor.matmul(
                            psp,
                            lhsT=wb_sb[0:1, brow * DH2 : (brow + 1) * DH2],
                            rhs=brhs[0:1, sl],
                            start=False,
                            stop=True,
                        )
                    evict(psp, sl)

            def evict_qk(dst):
                def _e(psp, sl):
                    for h in range(HPC):
                        nc.vector.tensor_copy(
                            out=dst[h][0:DH, sl], in_=psp[h * DH : (h + 1) * DH, :]
                        )

                return _e

            def b1_step(qh, h, pt, kt):
                st = ps.tile([128, QH], F32, tag="st", bufs=2, name="st")
                for c2 in range(QH // 512):
                    nc.tensor.matmul(
                        st[:, c2 * 512 : (c2 + 1) * 512],
                        lhsT=KE[h][0:66, kt * 128 : (kt + 1) * 128],
                        rhs=QE[h][
                            0:66, qh * QH + c2 * 512 : qh * QH + (c2 + 1) * 512
                        ],
                        start=True,
                        stop=True,
                    )
                nc.scalar.activation(
                    out=pt[:, kt, :], in_=st, func=mybir.ActivationFunctionType.Exp
                )

            def b1_steps(qh, h, pt):
                for kt in range(NKT):
                    yield lambda kt=kt: b1_step(qh, h, pt, kt)

            def b2_steps(qh, h, pt, outp):
                for kt in range(NKT):
                    def _s(kt=kt):
                        for c2 in range(QH // 512):
                            nc.tensor.matmul(
                                outp[:, c2 * 512 : (c2 + 1) * 512],
                                lhsT=Vp[h][:, kt, :],
                                rhs=pt[:, kt, c2 * 512 : (c2 + 1) * 512],
                                start=(kt == 0),
                                stop=(kt == NKT - 1),
                            )
                    yield _s

            def interleave(*gens):
                gens = [iter(g) for g in gens if g is not None]
                while gens:
                    nxt = []
                    for g in gens:
                        try:
                            next(g)()
                        except StopIteration:
                            continue
                        nxt.append(g)
                    gens = nxt

            def emit_norm(qh, h, outp):
                outsb = sbB.tile([65, QH], F32, tag="outsb", name="outsb")
                nc.vector.tensor_copy(out=outsb, in_=outp)
                rcp = sbB.tile([1, QH], F32, tag="rcp", name="rcp")
                nc.vector.reciprocal(out=rcp, in_=outsb[64:65, :])
                rbc = sbB.tile([DH, QH], F32, tag="rbc", name="rbc")
                nc.gpsimd.partition_broadcast(rbc, rcp[0:1, :], channels=DH)
                nr = sbB.tile([DH, QH], MM_DT, tag=f"nrm{h}", name=f"nr{h}")
                nc.vector.tensor_mul(out=nr, in0=outsb[0:DH, :], in1=rbc)
                return nr

            def emit_finals(qh, nrm):
                for dt4 in range(4):
                    for c2 in range(QH // 512):
                        fin = ps.tile([128, 512], F32, tag="small", bufs=2, name="fin")
                        for h in range(HPC):
                            nc.tensor.matmul(
                                fin,
                                lhsT=wo_sb[h][:, dt4 * 128 : (dt4 + 1) * 128],
                                rhs=nrm[h][:, c2 * 512 : (c2 + 1) * 512],
                                start=(h == 0),
                                stop=(h == HPC - 1),
                            )
                        fsb = sbB.tile([128, 512], F32, tag="fsb", bufs=2, name="fsb")
                        nc.vector.tensor_copy(out=fsb, in_=fin)
                        nc.sync.dma_start(
                            out=pout_d[
                                dt4 * 128 : (dt4 + 1) * 128,
                                qh * QH + c2 * 512 : qh * QH + (c2 + 1) * 512,
                            ],
                            in_=fsb,
                        )

            def vproj_steps():
                nc.sync.dma_start(
                    out=wv_sb, in_=wvs_d[:, :].rearrange("(t p) m -> p t m", p=128)
                )
                nc.sync.dma_start(
                    out=coef_sb, in_=coef_d[0:1, :].rearrange("1 (t p) -> p t", p=128)
                )
                VT_sb = vtmp.tile([128, L], F32)
                xts = []
                for t in range(4):
                    xt = xin.tile([128, L], MM_DT, tag="xin", name=f"xv{t}")
                    nc.sync.dma_start(out=xt, in_=xvT_d[t * 128 : (t + 1) * 128, :])
                    xts.append(xt)

                def _chunk(ch):
                    sl = slice(ch * 512, (ch + 1) * 512)
                    psp = ps.tile([128, 512], F32, tag="small", bufs=2, name="psp")
                    for t in range(4):
                        nc.tensor.matmul(
                            psp,
                            lhsT=wv_sb[:, t, :],
                            rhs=xts[t][:, sl],
                            start=(t == 0),
                            stop=(t == 3 and not with_bias),
                        )
                    if with_bias:
                        nc.tensor.matmul(
                            psp,
                            lhsT=wb_sb[0:1, 2 * DH2 : 3 * DH2],
                            rhs=ones_sb[0:1, sl],
                            start=False,
                            stop=True,
                        )
                    nc.vector.tensor_copy(out=VT_sb[:, sl], in_=psp)

                def _tp(kt):
                    tp = ps.tile([128, 128], F32, tag="small", bufs=2, name="tp")
                    nc.tensor.transpose(tp, VT_sb[:, kt * 128 : (kt + 1) * 128], ident)
                    for h in range(HPC):
                        nc.vector.tensor_scalar_mul(
                            out=Vp[h][:, kt, 0:DH],
                            in0=tp[:, h * DH : (h + 1) * DH],
                            scalar1=coef_sb[:, kt : kt + 1],
                        )

                for ch in range(4):
                    yield lambda ch=ch: _chunk(ch)
                for kt in range(NKT):
                    yield lambda kt=kt: _tp(kt)

            # ---- emission: software-pipelined over 4 attention units ----
            emit_proj("q", xqT_d, wq_sb, 0, qm_sb if with_bias else None, evict_qk(QE))
            nc.sync.dma_start(
                out=wk_sb, in_=wks_d[:, :].rearrange("(t p) m -> p t m", p=128)
            )
            emit_proj(
                "k", xkT_d, wk_sb, 1, ones_sb if with_bias else None, evict_qk(KE)
            )

            for h in range(HPC):
                nc.sync.dma_start(out=wo_sb[h], in_=wos_d[h * DH : (h + 1) * DH, :])

            units = [(0, 0), (0, 1), (1, 0), (1, 1)]
            pts = {}
            outps = {}
            nrms = {}
            # unit 0 scores interleaved with the v projection/transpose
            pts[0] = ptp.tile([128, NKT, QH], ATT_DT, tag="pt", name="pt0")
            interleave(b1_steps(0, 0, pts[0]), vproj_steps())
            for i in range(1, 4):
                qh, h = units[i]
                pqh, ph = units[i - 1]
                pts[i] = ptp.tile([128, NKT, QH], ATT_DT, tag="pt", name=f"pt{i}")
                outps[i - 1] = ps.tile([65, QH], F32, tag="outp", bufs=1, name="outp")
                interleave(
                    b1_steps(qh, h, pts[i]),
                    b2_steps(pqh, ph, pts[i - 1], outps[i - 1]),
                )
                nrms[i - 1] = emit_norm(pqh, ph, outps[i - 1])
                if i == 2:
                    emit_finals(0, [nrms[0], nrms[1]])
            outps[3] = ps.tile([65, QH], F32, tag="outp", bufs=1, name="outp")
            for s in b2_steps(1, 1, pts[3], outps[3]):
                s()
            nrms[3] = emit_norm(1, 1, outps[3])
            emit_finals(1, [nrms[2], nrms[3]])

    nc.compile()
    return nc


_CACHE = {}


def _get_nc(with_bias: bool):
    key = ("nc", with_bias)
    if key not in _CACHE:
        _CACHE[key] = build_nc(with_bias)
    return _CACHE[key]


def kernel(q, k, v, text_mask, audio_mask, n_head, wq, bq, wk, bk, wv, bv, wo, bo):
    global LAST_RESULT
    q = np.asarray(q, np.float32)
    k = np.asarray(k, np.float32)
    v = np.asarray(v, np.float32)
    text_mask = np.asarray(text_mask, np.float32)
    audio_mask = np.asarray(audio_mask, np.float32)
    wq = np.asarray(wq, np.float32)
    wk = np.asarray(wk, np.float32)
    wv = np.asarray(wv, np.float32)
    wo = np.asarray(wo, np.float32)
    bq = np.asarray(bq, np.float32)
    bk = np.asarray(bk, np.float32)
    bv = np.asarray(bv, np.float32)
    bo = np.asarray(bo, np.float32)
    assert int(n_head) == H

    with_bias = bool(np.any(bq) or np.any(bk) or np.any(bv))

    pad = np.concatenate([text_mask, audio_mask], axis=1)  # [B, L]
    qm = (pad != 0).astype(np.float32)
    tl = text_mask.sum(1)
    al = audio_mask.sum(1)
    tot = tl + al
    coef = np.concatenate(
        [
            text_mask * (tot / (2.0 * tl))[:, None],
            audio_mask * (tot / (2.0 * al))[:, None],
        ],
        axis=1,
    ).astype(np.float32)
    kbmc = (NEG * (1.0 - qm) - C_LN).astype(np.float32)
    ones_row = np.ones((L,), np.float32)
    cln_row = np.full((L,), C_LN, np.float32)

    def cc(a):
        return np.ascontiguousarray(a, dtype=np.float32)

    in_maps = []
    for core in range(NCORES):
        b, hp = divmod(core, NCORES // B)
        cols = slice(hp * DH2, (hp + 1) * DH2)
        m = {
            "xqT": cc((q[b] * qm[b][:, None]).T),
            "xkT": cc(k[b].T),
            "xvT": cc(v[b].T),
            "wqs": cc(wq.T[:, cols]),
            "wks": cc(wk.T[:, cols] / 8.0),
            "wvs": cc(wv.T[:, cols]),
            "wos": cc(wo.T[cols, :]),
            "aux": cc(np.stack([qm[b], kbmc[b], ones_row, cln_row])),
            "coef": cc(coef[b]).reshape(1, L),
        }
        if with_bias:
            m["wbias"] = cc(
                np.concatenate(
                    [bq[cols], bk[cols] / 8.0, bv[cols], np.zeros(DH2, np.float32)]
                )
            ).reshape(1, 4 * DH2)
        in_maps.append(m)

    res = run_bass_kernel_spmd(
        _get_nc(with_bias), in_maps, core_ids=list(range(NCORES)), trace=TRACE
    )
    LAST_RESULT = res

    out = np.zeros((B, L, DM), np.float32)
    npc = NCORES // B
    for b in range(B):
        acc = res.results[b * npc]["poutT"].astype(np.float32).copy()
        for hp in range(1, npc):
            acc += res.results[b * npc + hp]["poutT"]
        out[b] = acc.T + bo[None, :]
    return out



# revision 3
# speedup vs baseline: 1.1745x; 1.1745x over previous
"""Trainium2 Bass kernel for CustomMultiHeadAttention (B=2, L=2048, D=512, H=8).

Sharding: 8 cores = 2 batches x 4 head-pairs. Each core computes, for its
batch b and its 2 heads, the partial output (O_h @ Wo_h summed over its
heads), transposed: poutT [512, 2048]. Host sums the 4 partials per batch,
transposes, and adds bo.

Device-side math per core (all masking folded into matmul contractions):
  Qh = (q[b]*qm) @ WqT_cols + qm*bq_cols          (masked q rows -> exactly 0)
  Kh = k[b] @ WkT_cols/8 + bk_cols/8
  E[k,q] = Kh.Qh + (kb[k]-c)*qm[q] + c            via 2 extra contraction rows
           (kb = -1e4 for masked keys, c = ln(1/2048))
    -> unmasked q: E = s + kb  (masked keys underflow to 0 in exp)
    -> masked q:   E = c       (exp = 1/2048 uniform; denom = 1)
  PT = exp(E)   [k, q] layout
  outT = [Vp | 1]^T @ PT   (Vp = coef * Vh; ones column yields denom row)
  O = outT[0:64] / outT[64]
  poutT[d, q] += Wo[:, d] . O2[:, q]   (both heads packed, contraction 128)

Perf notes vs the first version:
  - x and weights travel as bf16 (halves input DMA), x in [128,512] blocks
    chunk-major so the first projection matmul starts ~2us in.
  - PE warm-up matmuls at t=0 ride the DVFS ramp (0.65->2.4GHz after ~3us
    of continuous busy).
  - Output projection packs both heads into contraction-128 matmuls.
  - reciprocal_approx_fast (~5x faster than reciprocal); norm + finals are
    chunked at 512 cols so the tail pipelines instead of serializing.
  - poutT returns as bf16 (halves output DMA).
"""

import math
import os

os.environ.setdefault("MYCRO_LOCAL_CACHE", "1")

import numpy as np

import concourse.bass as bass
import concourse.tile as tile
from concourse import bacc
from concourse import mybir
from concourse.bass_utils import run_bass_kernel_spmd
from concourse.masks import make_identity

B = 2
L = 2048
DM = 512
H = 8
DH = 64
NCORES = 8
HPC = 2           # heads per core
DH2 = HPC * DH    # 128
NKT = L // 128    # 16 k tiles
QH = 1024         # q chunk for attention phase
NQH = L // QH     # 2
C_LN = -math.log(L)
NEG = -10000.0

F32 = mybir.dt.float32
F32R = mybir.dt.float32r
BF16 = mybir.dt.bfloat16

ATT_DT = BF16     # exp output (PT), V'
QK_DT = F32R      # QE/KE on-chip operands (full-rate, ~tf32 rounding)
X_DT = BF16       # x / w wire + projection matmul dtype

TRACE = False
LAST_RESULT = None

AUX_QM, AUX_KBMC, AUX_ONES, AUX_CLN = 0, 1, 2, 3


def build_nc(with_bias: bool):
    nc = bacc.Bacc(None, target_bir_lowering=False)

    xqT_d = nc.declare_dram_parameter("xqT", [DM, L], X_DT, isOutput=False)
    xkT_d = nc.declare_dram_parameter("xkT", [DM, L], X_DT, isOutput=False)
    xvT_d = nc.declare_dram_parameter("xvT", [DM, L], X_DT, isOutput=False)
    wqs_d = nc.declare_dram_parameter("wqs", [DM, DH2], X_DT, isOutput=False)
    wks_d = nc.declare_dram_parameter("wks", [DM, DH2], X_DT, isOutput=False)
    wvs_d = nc.declare_dram_parameter("wvs", [DM, DH2], X_DT, isOutput=False)
    if with_bias:
        wbias_d = nc.declare_dram_parameter(
            "wbias", [1, 4 * DH2], X_DT, isOutput=False
        )
    wos_d = nc.declare_dram_parameter("wos", [DH2, DM], BF16, isOutput=False)
    aux_d = nc.declare_dram_parameter("aux", [4, L], QK_DT, isOutput=False)
    coef_d = nc.declare_dram_parameter("coef", [1, L], F32, isOutput=False)
    pout_d = nc.declare_dram_parameter("poutT", [DM, L], BF16, isOutput=True)

    with tile.TileContext(nc) as tc:
        with (
            tc.tile_pool(name="const", bufs=1) as const,
            tc.tile_pool(name="qek", bufs=1) as qek,
            tc.tile_pool(name="xin", bufs=1) as xin,
            tc.tile_pool(name="vtmp", bufs=1) as vtmp,
            tc.tile_pool(name="ptp", bufs=2) as ptp,
            tc.tile_pool(name="sbB", bufs=1) as sbB,
            tc.tile_pool(name="ps", bufs=1, space="PSUM") as ps,
        ):
            # ---- PE warm-up: ride the DVFS ramp while input DMAs land ----
            wrm = const.tile([128, 512], X_DT)
            nc.vector.memset(wrm, 1.0)
            for w in range(7):
                pw = ps.tile([128, 512], F32, tag="small", bufs=2, name=f"warm{w}")
                nc.tensor.matmul(
                    pw, lhsT=wrm[:, 0:128], rhs=wrm, start=True, stop=True
                )

            # ---- constants ----
            ident = const.tile([128, 128], F32)
            make_identity(nc, ident)
            wq_sb = const.tile([128, 4, DH2], X_DT)
            nc.sync.dma_start(
                out=wq_sb, in_=wqs_d[:, :].rearrange("(t p) m -> p t m", p=128)
            )
            wk_sb = const.tile([128, 4, DH2], X_DT)
            nc.sync.dma_start(
                out=wk_sb, in_=wks_d[:, :].rearrange("(t p) m -> p t m", p=128)
            )
            wv_sb = const.tile([128, 4, DH2], X_DT)
            wo_sb = const.tile([DH2, DM], BF16)
            coef_sb = const.tile([128, NKT], F32)
            if with_bias:
                wb_sb = const.tile([1, 4 * DH2], X_DT)
                nc.sync.dma_start(out=wb_sb, in_=wbias_d[:, :])
                qm_sb = const.tile([1, L], X_DT)
                ones_sb = const.tile([1, L], X_DT)
                nc.vector.memset(ones_sb, 1.0)

            # ---- x input blocks: [128, 512], chunk-major so chunk 0 of a
            # projection is computable after 4 small DMAs ----
            def x_blocks(pname, xdram):
                blocks = []
                for ch in range(4):
                    col = []
                    for t in range(4):
                        xt = xin.tile(
                            [128, 512], X_DT, tag="xin", bufs=48,
                            name=f"x{pname}{t}_{ch}",
                        )
                        nc.sync.dma_start(
                            out=xt,
                            in_=xdram[
                                t * 128 : (t + 1) * 128,
                                ch * 512 : (ch + 1) * 512,
                            ],
                        )
                        col.append(xt)
                    blocks.append(col)
                return blocks

            xq_b = x_blocks("q", xqT_d)
            xk_b = x_blocks("k", xkT_d)
            if with_bias:
                # qm row as X_DT for the bias matmul rhs
                nc.sync.dma_start(out=qm_sb, in_=aux_d[AUX_QM : AUX_QM + 1, :])

            # ---- persistent per-head operands ----
            QE = [qek.tile([66, L], QK_DT, name=f"QE{h}") for h in range(HPC)]
            KE = [qek.tile([66, L], QK_DT, name=f"KE{h}") for h in range(HPC)]
            Vp = [
                qek.tile([128, NKT, DH + 1], ATT_DT, name=f"Vp{h}") for h in range(HPC)
            ]
            # mask/bias rows of the extended operands (DMA direct from host aux)
            for h in range(HPC):
                nc.sync.dma_start(
                    out=QE[h][64:65, :], in_=aux_d[AUX_QM : AUX_QM + 1, :]
                )
                nc.sync.dma_start(
                    out=QE[h][65:66, :], in_=aux_d[AUX_ONES : AUX_ONES + 1, :]
                )
                nc.sync.dma_start(
                    out=KE[h][64:65, :], in_=aux_d[AUX_KBMC : AUX_KBMC + 1, :]
                )
                nc.sync.dma_start(
                    out=KE[h][65:66, :], in_=aux_d[AUX_CLN : AUX_CLN + 1, :]
                )
                nc.vector.memset(Vp[h][:, :, DH : DH + 1], 1.0)

            def emit_proj(pname, xb, w_sb, brow, brhs, evict):
                for ch in range(4):
                    sl = slice(ch * 512, (ch + 1) * 512)
                    psp = ps.tile([128, 512], F32, tag="small", bufs=2, name="psp")
                    for t in range(4):
                        nc.tensor.matmul(
                            psp,
                            lhsT=w_sb[:, t, :],
                            rhs=xb[ch][t],
                            start=(t == 0),
                            stop=(t == 3 and not with_bias),
                        )
                    if with_bias:
                        nc.tensor.matmul(
                            psp,
                            lhsT=wb_sb[0:1, brow * DH2 : (brow + 1) * DH2],
                            rhs=brhs[0:1, sl],
                            start=False,
                            stop=True,
                        )
                    evict(psp, sl)

            def evict_qk(dst):
                def _e(psp, sl):
                    for h in range(HPC):
                        nc.vector.tensor_copy(
                            out=dst[h][0:DH, sl], in_=psp[h * DH : (h + 1) * DH, :]
                        )

                return _e

            def b1_step(qh, h, pt, kt):
                st = ps.tile([128, QH], F32, tag="st", bufs=2, name="st")
                for c2 in range(QH // 512):
                    nc.tensor.matmul(
                        st[:, c2 * 512 : (c2 + 1) * 512],
                        lhsT=KE[h][0:66, kt * 128 : (kt + 1) * 128],
                        rhs=QE[h][
                            0:66, qh * QH + c2 * 512 : qh * QH + (c2 + 1) * 512
                        ],
                        start=True,
                        stop=True,
                    )
                nc.scalar.activation(
                    out=pt[:, kt, :], in_=st, func=mybir.ActivationFunctionType.Exp
                )

            def b1_steps(qh, h, pt):
                for kt in range(NKT):
                    yield lambda kt=kt: b1_step(qh, h, pt, kt)

            def b2_steps(qh, h, pt, outp):
                for kt in range(NKT):
                    def _s(kt=kt):
                        for c2 in range(QH // 512):
                            nc.tensor.matmul(
                                outp[:, c2 * 512 : (c2 + 1) * 512],
                                lhsT=Vp[h][:, kt, :],
                                rhs=pt[:, kt, c2 * 512 : (c2 + 1) * 512],
                                start=(kt == 0),
                                stop=(kt == NKT - 1),
                            )
                    yield _s

            def interleave(*gens):
                gens = [iter(g) for g in gens if g is not None]
                while gens:
                    nxt = []
                    for g in gens:
                        try:
                            next(g)()
                        except StopIteration:
                            continue
                        nxt.append(g)
                    gens = nxt

            def emit_norm_chunk(h, outp, nrm2, c2):
                sl = slice(c2 * 512, (c2 + 1) * 512)
                outsb = sbB.tile([DH, 512], F32, tag="outsb", bufs=2, name="outsb")
                nc.vector.tensor_copy(out=outsb, in_=outp[0:DH, sl])
                den = sbB.tile([1, 512], F32, tag="den", bufs=2, name="den")
                nc.vector.tensor_copy(out=den, in_=outp[DH : DH + 1, sl])
                rcp = sbB.tile([1, 512], F32, tag="rcp", bufs=2, name="rcp")
                nc.vector.reciprocal_approx_fast(out=rcp, in_=den)
                rbc = sbB.tile([DH, 512], F32, tag="rbc", bufs=2, name="rbc")
                nc.gpsimd.partition_broadcast(rbc, rcp[0:1, :], channels=DH)
                nc.vector.tensor_mul(
                    out=nrm2[h * DH : (h + 1) * DH, sl], in0=outsb, in1=rbc
                )

            def emit_norm(h, outp, nrm2):
                for c2 in range(QH // 512):
                    emit_norm_chunk(h, outp, nrm2, c2)

            def emit_finals_chunk(qh, nrm2, c2):
                for dt4 in range(4):
                    fin = ps.tile([128, 512], F32, tag="small", bufs=2, name="fin")
                    nc.tensor.matmul(
                        fin,
                        lhsT=wo_sb[:, dt4 * 128 : (dt4 + 1) * 128],
                        rhs=nrm2[:, c2 * 512 : (c2 + 1) * 512],
                        start=True,
                        stop=True,
                    )
                    fsb = sbB.tile([128, 512], BF16, tag="fsb", bufs=2, name="fsb")
                    nc.vector.tensor_copy(out=fsb, in_=fin)
                    nc.sync.dma_start(
                        out=pout_d[
                            dt4 * 128 : (dt4 + 1) * 128,
                            qh * QH + c2 * 512 : qh * QH + (c2 + 1) * 512,
                        ],
                        in_=fsb,
                    )

            def vproj_steps():
                nc.sync.dma_start(
                    out=wv_sb, in_=wvs_d[:, :].rearrange("(t p) m -> p t m", p=128)
                )
                nc.sync.dma_start(
                    out=coef_sb, in_=coef_d[0:1, :].rearrange("1 (t p) -> p t", p=128)
                )
                nc.sync.dma_start(out=wo_sb, in_=wos_d[:, :])
                VT_sb = vtmp.tile([128, L], F32)
                xv_b = x_blocks("v", xvT_d)

                def _chunk(ch):
                    sl = slice(ch * 512, (ch + 1) * 512)
                    psp = ps.tile([128, 512], F32, tag="small", bufs=2, name="psp")
                    for t in range(4):
                        nc.tensor.matmul(
                            psp,
                            lhsT=wv_sb[:, t, :],
                            rhs=xv_b[ch][t],
                            start=(t == 0),
                            stop=(t == 3 and not with_bias),
                        )
                    if with_bias:
                        nc.tensor.matmul(
                            psp,
                            lhsT=wb_sb[0:1, 2 * DH2 : 3 * DH2],
                            rhs=ones_sb[0:1, sl],
                            start=False,
                            stop=True,
                        )
                    nc.vector.tensor_copy(out=VT_sb[:, sl], in_=psp)

                def _tp(kt):
                    tp = ps.tile([128, 128], F32, tag="small", bufs=2, name="tp")
                    nc.tensor.transpose(tp, VT_sb[:, kt * 128 : (kt + 1) * 128], ident)
                    for h in range(HPC):
                        nc.vector.tensor_scalar_mul(
                            out=Vp[h][:, kt, 0:DH],
                            in0=tp[:, h * DH : (h + 1) * DH],
                            scalar1=coef_sb[:, kt : kt + 1],
                        )

                for ch in range(4):
                    yield lambda ch=ch: _chunk(ch)
                for kt in range(NKT):
                    yield lambda kt=kt: _tp(kt)

            # ---- emission: software-pipelined over 4 attention units ----
            emit_proj("q", xq_b, wq_sb, 0, qm_sb if with_bias else None, evict_qk(QE))
            emit_proj(
                "k", xk_b, wk_sb, 1, ones_sb if with_bias else None, evict_qk(KE)
            )

            units = [(0, 0), (0, 1), (1, 0), (1, 1)]
            pts = {}
            outps = {}
            nrm2s = {
                0: sbB.tile([DH2, QH], BF16, tag="nrm", bufs=2, name="nrm2_0"),
                1: sbB.tile([DH2, QH], BF16, tag="nrm", bufs=2, name="nrm2_1"),
            }
            # unit 0 scores interleaved with the v projection/transpose
            pts[0] = ptp.tile([128, NKT, QH], ATT_DT, tag="pt", name="pt0")
            interleave(b1_steps(0, 0, pts[0]), vproj_steps())
            for i in range(1, 4):
                qh, h = units[i]
                pqh, ph = units[i - 1]
                pts[i] = ptp.tile([128, NKT, QH], ATT_DT, tag="pt", name=f"pt{i}")
                outps[i - 1] = ps.tile([65, QH], F32, tag="outp", bufs=1, name="outp")
                interleave(
                    b1_steps(qh, h, pts[i]),
                    b2_steps(pqh, ph, pts[i - 1], outps[i - 1]),
                )
                emit_norm(ph, outps[i - 1], nrm2s[pqh])
                if i == 2:
                    for c2 in range(QH // 512):
                        emit_finals_chunk(0, nrm2s[0], c2)
            # ---- tail: unit 3 b2 chunk-major, norm/finals pipelined ----
            outp3 = ps.tile([65, QH], F32, tag="outp", bufs=1, name="outp")
            for c2 in range(QH // 512):
                for kt in range(NKT):
                    nc.tensor.matmul(
                        outp3[:, c2 * 512 : (c2 + 1) * 512],
                        lhsT=Vp[1][:, kt, :],
                        rhs=pts[3][:, kt, c2 * 512 : (c2 + 1) * 512],
                        start=(kt == 0),
                        stop=(kt == NKT - 1),
                    )
                emit_norm_chunk(1, outp3, nrm2s[1], c2)
            for c2 in range(QH // 512):
                emit_finals_chunk(1, nrm2s[1], c2)

    nc.compile()
    return nc


_CACHE = {}


def _get_nc(with_bias: bool):
    key = ("nc", with_bias)
    if key not in _CACHE:
        _CACHE[key] = build_nc(with_bias)
    return _CACHE[key]


def kernel(q, k, v, text_mask, audio_mask, n_head, wq, bq, wk, bk, wv, bv, wo, bo):
    global LAST_RESULT
    import ml_dtypes

    bf16 = ml_dtypes.bfloat16

    q = np.asarray(q, np.float32)
    k = np.asarray(k, np.float32)
    v = np.asarray(v, np.float32)
    text_mask = np.asarray(text_mask, np.float32)
    audio_mask = np.asarray(audio_mask, np.float32)
    wq = np.asarray(wq, np.float32)
    wk = np.asarray(wk, np.float32)
    wv = np.asarray(wv, np.float32)
    wo = np.asarray(wo, np.float32)
    bq = np.asarray(bq, np.float32)
    bk = np.asarray(bk, np.float32)
    bv = np.asarray(bv, np.float32)
    bo = np.asarray(bo, np.float32)
    assert int(n_head) == H

    with_bias = bool(np.any(bq) or np.any(bk) or np.any(bv))

    pad = np.concatenate([text_mask, audio_mask], axis=1)  # [B, L]
    qm = (pad != 0).astype(np.float32)
    tl = text_mask.sum(1)
    al = audio_mask.sum(1)
    tot = tl + al
    coef = np.concatenate(
        [
            text_mask * (tot / (2.0 * tl))[:, None],
            audio_mask * (tot / (2.0 * al))[:, None],
        ],
        axis=1,
    ).astype(np.float32)
    kbmc = (NEG * (1.0 - qm) - C_LN).astype(np.float32)
    ones_row = np.ones((L,), np.float32)
    cln_row = np.full((L,), C_LN, np.float32)

    def cb(a):
        return np.ascontiguousarray(np.asarray(a, np.float32).astype(bf16))

    def cc(a):
        return np.ascontiguousarray(a, dtype=np.float32)

    in_maps = []
    for core in range(NCORES):
        b, hp = divmod(core, NCORES // B)
        cols = slice(hp * DH2, (hp + 1) * DH2)
        m = {
            "xqT": cb((q[b] * qm[b][:, None]).T),
            "xkT": cb(k[b].T),
            "xvT": cb(v[b].T),
            "wqs": cb(wq.T[:, cols]),
            "wks": cb(wk.T[:, cols] / 8.0),
            "wvs": cb(wv.T[:, cols]),
            "wos": cb(wo.T[cols, :]),
            "aux": cc(np.stack([qm[b], kbmc[b], ones_row, cln_row])),
            "coef": cc(coef[b]).reshape(1, L),
        }
        if with_bias:
            m["wbias"] = cb(
                np.concatenate(
                    [bq[cols], bk[cols] / 8.0, bv[cols], np.zeros(DH2, np.float32)]
                )
            ).reshape(1, 4 * DH2)
        in_maps.append(m)

    res = run_bass_kernel_spmd(
        _get_nc(with_bias), in_maps, core_ids=list(range(NCORES)), trace=TRACE
    )
    LAST_RESULT = res

    out = np.zeros((B, L, DM), np.float32)
    npc = NCORES // B
    for b in range(B):
        acc = res.results[b * npc]["poutT"].astype(np.float32)
        for hp in range(1, npc):
            acc = acc + res.results[b * npc + hp]["poutT"].astype(np.float32)
        out[b] = acc.T + bo[None, :]
    return out


# revision 7
# speedup vs baseline: 1.1751x; 1.0005x over previous
"""Trainium2 Bass kernel for CustomMultiHeadAttention (B=2, L=2048, D=512, H=8).

Sharding: 8 cores = 2 batches x 4 head-pairs. Each core computes, for its
batch b and its 2 heads, the partial output (O_h @ Wo_h summed over its
heads), transposed: poutT [512, 2048]. Host sums the 4 partials per batch,
transposes, and adds bo.

Device-side math per core (all masking folded into matmul contractions):
  Qh = (q[b]*qm) @ WqT_cols + qm*bq_cols          (masked q rows -> exactly 0)
  Kh = k[b] @ WkT_cols/8 + bk_cols/8
  E[k,q] = Kh.Qh + (kb[k]-c)*qm[q] + c            via 2 extra contraction rows
           (kb = -1e4 for masked keys, c = ln(1/2048))
    -> unmasked q: E = s + kb  (masked keys underflow to 0 in exp)
    -> masked q:   E = c       (exp = 1/2048 uniform; denom = 1)
  PT = exp(E)   [k, q] layout
  outT = [Vp | 1]^T @ PT   (Vp = coef * Vh; ones column yields denom row)
  O = outT[0:64] / outT[64]
  poutT[d, q] += Wo[:, d] . O2[:, q]   (both heads packed, contraction 128)

Perf notes vs the first version:
  - x and weights travel as bf16 (halves input DMA), x in [128,512] blocks
    chunk-major so the first projection matmul starts ~2us in.
  - PE warm-up matmuls at t=0 ride the DVFS ramp (0.65->2.4GHz after ~3us
    of continuous busy).
  - Output projection packs both heads into contraction-128 matmuls.
  - reciprocal_approx_fast (~5x faster than reciprocal); norm + finals are
    chunked at 512 cols so the tail pipelines instead of serializing.
  - poutT returns as bf16 (halves output DMA).
"""

import math
import os

os.environ.setdefault("MYCRO_LOCAL_CACHE", "1")

import numpy as np

import concourse.bass as bass
import concourse.tile as tile
from concourse import bacc
from concourse import mybir
from concourse.bass_utils import run_bass_kernel_spmd
from concourse.masks import make_identity

B = 2
L = 2048
DM = 512
H = 8
DH = 64
NCORES = 8
HPC = 2           # heads per core
DH2 = HPC * DH    # 128
NKT = L // 128    # 16 k tiles
QH = 1024         # q chunk for attention phase
NQH = L // QH     # 2
C_LN = -math.log(L)
NEG = -10000.0

F32 = mybir.dt.float32
F32R = mybir.dt.float32r
BF16 = mybir.dt.bfloat16

ATT_DT = BF16     # exp output (PT), V'
QK_DT = F32R      # QE/KE on-chip operands (full-rate, ~tf32 rounding)
X_DT = BF16       # x / w wire + projection matmul dtype

TRACE = False
LAST_RESULT = None

AUX_QM, AUX_KBMC, AUX_ONES, AUX_CLN = 0, 1, 2, 3


def build_nc(with_bias: bool):
    nc = bacc.Bacc(None, target_bir_lowering=False)

    xqT_d = nc.declare_dram_parameter("xqT", [DM, L], X_DT, isOutput=False)
    xkT_d = nc.declare_dram_parameter("xkT", [DM, L], X_DT, isOutput=False)
    xvT_d = nc.declare_dram_parameter("xvT", [DM, L], X_DT, isOutput=False)
    wqs_d = nc.declare_dram_parameter("wqs", [DM, DH2], X_DT, isOutput=False)
    wks_d = nc.declare_dram_parameter("wks", [DM, DH2], X_DT, isOutput=False)
    wvs_d = nc.declare_dram_parameter("wvs", [DM, DH2], X_DT, isOutput=False)
    if with_bias:
        wbias_d = nc.declare_dram_parameter(
            "wbias", [1, 4 * DH2], X_DT, isOutput=False
        )
    wos_d = nc.declare_dram_parameter("wos", [DH2, DM], BF16, isOutput=False)
    aux_d = nc.declare_dram_parameter("aux", [4, L], QK_DT, isOutput=False)
    coef_d = nc.declare_dram_parameter("coef", [1, L], F32, isOutput=False)
    pout_d = nc.declare_dram_parameter("poutT", [DM, L], BF16, isOutput=True)

    with tile.TileContext(nc) as tc:
        with (
            tc.tile_pool(name="const", bufs=1) as const,
            tc.tile_pool(name="qek", bufs=1) as qek,
            tc.tile_pool(name="xin", bufs=1) as xin,
            tc.tile_pool(name="vtmp", bufs=1) as vtmp,
            tc.tile_pool(name="ptp", bufs=2) as ptp,
            tc.tile_pool(name="sbB", bufs=1) as sbB,
            tc.tile_pool(name="ps", bufs=1, space="PSUM") as ps,
        ):
            # ---- PE warm-up: ride the DVFS ramp while input DMAs land ----
            wrm = const.tile([128, 512], X_DT)
            nc.vector.memset(wrm, 1.0)
            for w in range(7):
                pw = ps.tile([128, 512], F32, tag="small", bufs=2, name=f"warm{w}")
                nc.tensor.matmul(
                    pw, lhsT=wrm[:, 0:128], rhs=wrm, start=True, stop=True
                )

            # ---- constants ----
            ident = const.tile([128, 128], F32)
            make_identity(nc, ident)
            wq_sb = const.tile([128, 4, DH2], X_DT)
            nc.sync.dma_start(
                out=wq_sb, in_=wqs_d[:, :].rearrange("(t p) m -> p t m", p=128)
            )
            wk_sb = const.tile([128, 4, DH2], X_DT)
            nc.sync.dma_start(
                out=wk_sb, in_=wks_d[:, :].rearrange("(t p) m -> p t m", p=128)
            )
            wv_sb = const.tile([128, 4, DH2], X_DT)
            wo_sb = const.tile([DH2, DM], BF16)
            coef_sb = const.tile([128, NKT], F32)
            if with_bias:
                wb_sb = const.tile([1, 4 * DH2], X_DT)
                nc.sync.dma_start(out=wb_sb, in_=wbias_d[:, :])
                qm_sb = const.tile([1, L], X_DT)
                ones_sb = const.tile([1, L], X_DT)
                nc.vector.memset(ones_sb, 1.0)

            # ---- x input blocks: [128, 512], chunk-major so chunk 0 of a
            # projection is computable after 4 small DMAs ----
            def x_blocks(pname, xdram):
                blocks = []
                for ch in range(4):
                    col = []
                    for t in range(4):
                        xt = xin.tile(
                            [128, 512], X_DT, tag="xin", bufs=48,
                            name=f"x{pname}{t}_{ch}",
                        )
                        nc.sync.dma_start(
                            out=xt,
                            in_=xdram[
                                t * 128 : (t + 1) * 128,
                                ch * 512 : (ch + 1) * 512,
                            ],
                        )
                        col.append(xt)
                    blocks.append(col)
                return blocks

            xq_b = x_blocks("q", xqT_d)
            xk_b = x_blocks("k", xkT_d)
            if with_bias:
                # qm row as X_DT for the bias matmul rhs
                nc.sync.dma_start(out=qm_sb, in_=aux_d[AUX_QM : AUX_QM + 1, :])

            # ---- persistent per-head operands ----
            QE = [qek.tile([66, L], QK_DT, name=f"QE{h}") for h in range(HPC)]
            KE = [qek.tile([66, L], QK_DT, name=f"KE{h}") for h in range(HPC)]
            Vp = [
                qek.tile([128, NKT, DH + 1], ATT_DT, name=f"Vp{h}") for h in range(HPC)
            ]
            # mask/bias rows of the extended operands (DMA direct from host aux)
            for h in range(HPC):
                nc.sync.dma_start(
                    out=QE[h][64:65, :], in_=aux_d[AUX_QM : AUX_QM + 1, :]
                )
                nc.sync.dma_start(
                    out=QE[h][65:66, :], in_=aux_d[AUX_ONES : AUX_ONES + 1, :]
                )
                nc.sync.dma_start(
                    out=KE[h][64:65, :], in_=aux_d[AUX_KBMC : AUX_KBMC + 1, :]
                )
                nc.sync.dma_start(
                    out=KE[h][65:66, :], in_=aux_d[AUX_CLN : AUX_CLN + 1, :]
                )
                nc.vector.memset(Vp[h][:, :, DH : DH + 1], 1.0)

            def emit_proj(pname, xb, w_sb, brow, brhs, evict):
                for ch in range(4):
                    sl = slice(ch * 512, (ch + 1) * 512)
                    psp = ps.tile([128, 512], F32, tag="small", bufs=2, name="psp")
                    for t in range(4):
                        nc.tensor.matmul(
                            psp,
                            lhsT=w_sb[:, t, :],
                            rhs=xb[ch][t],
                            start=(t == 0),
                            stop=(t == 3 and not with_bias),
                        )
                    if with_bias:
                        nc.tensor.matmul(
                            psp,
                            lhsT=wb_sb[0:1, brow * DH2 : (brow + 1) * DH2],
                            rhs=brhs[0:1, sl],
                            start=False,
                            stop=True,
                        )
                    evict(psp, sl)

            def evict_qk(dst):
                def _e(psp, sl):
                    for h in range(HPC):
                        nc.vector.tensor_copy(
                            out=dst[h][0:DH, sl], in_=psp[h * DH : (h + 1) * DH, :]
                        )

                return _e

            def b1_step(qh, h, pt, kt):
                st = ps.tile([128, QH], F32, tag="st", bufs=2, name="st")
                for c2 in range(QH // 512):
                    nc.tensor.matmul(
                        st[:, c2 * 512 : (c2 + 1) * 512],
                        lhsT=KE[h][0:66, kt * 128 : (kt + 1) * 128],
                        rhs=QE[h][
                            0:66, qh * QH + c2 * 512 : qh * QH + (c2 + 1) * 512
                        ],
                        start=True,
                        stop=True,
                    )
                nc.scalar.activation(
                    out=pt[:, kt, :], in_=st, func=mybir.ActivationFunctionType.Exp
                )

            def b1_steps(qh, h, pt):
                for kt in range(NKT):
                    yield lambda kt=kt: b1_step(qh, h, pt, kt)

            def b2_steps(qh, h, pt, outp):
                for kt in range(NKT):
                    def _s(kt=kt):
                        for c2 in range(QH // 512):
                            nc.tensor.matmul(
                                outp[:, c2 * 512 : (c2 + 1) * 512],
                                lhsT=Vp[h][:, kt, :],
                                rhs=pt[:, kt, c2 * 512 : (c2 + 1) * 512],
                                start=(kt == 0),
                                stop=(kt == NKT - 1),
                            )
                    yield _s

            def interleave(*gens):
                gens = [iter(g) for g in gens if g is not None]
                while gens:
                    nxt = []
                    for g in gens:
                        try:
                            next(g)()
                        except StopIteration:
                            continue
                        nxt.append(g)
                    gens = nxt

            from concourse.alu_op_type import AluOpType

            def emit_norm_chunk(h, outp, nrm2, c2):
                sl = slice(c2 * 512, (c2 + 1) * 512)
                den = sbB.tile([1, 512], F32, tag="den", bufs=2, name="den")
                nc.vector.tensor_copy(out=den, in_=outp[DH : DH + 1, sl])
                rcp = sbB.tile([1, 512], F32, tag="rcp", bufs=2, name="rcp")
                nc.vector.reciprocal_approx_fast(out=rcp, in_=den)
                rbc = sbB.tile([DH, 512], F32, tag="rbc", bufs=2, name="rbc")
                nc.gpsimd.partition_broadcast(rbc, rcp[0:1, :], channels=DH)
                # nrm2 = (outp * 1.0) * rbc  — fused PSUM read + scale
                nc.vector.scalar_tensor_tensor(
                    out=nrm2[h * DH : (h + 1) * DH, sl],
                    in0=outp[0:DH, sl],
                    scalar=1.0,
                    in1=rbc,
                    op0=AluOpType.mult,
                    op1=AluOpType.mult,
                )

            def emit_norm(h, outp, nrm2):
                for c2 in range(QH // 512):
                    emit_norm_chunk(h, outp, nrm2, c2)

            def emit_finals_chunk(qh, nrm2, c2):
                for dt4 in range(4):
                    fin = ps.tile([128, 512], F32, tag="small", bufs=2, name="fin")
                    nc.tensor.matmul(
                        fin,
                        lhsT=wo_sb[:, dt4 * 128 : (dt4 + 1) * 128],
                        rhs=nrm2[:, c2 * 512 : (c2 + 1) * 512],
                        start=True,
                        stop=True,
                    )
                    fsb = sbB.tile([128, 512], BF16, tag="fsb", bufs=3, name="fsb")
                    nc.scalar.activation(
                        out=fsb, in_=fin,
                        func=mybir.ActivationFunctionType.Copy,
                    )
                    nc.sync.dma_start(
                        out=pout_d[
                            dt4 * 128 : (dt4 + 1) * 128,
                            qh * QH + c2 * 512 : qh * QH + (c2 + 1) * 512,
                        ],
                        in_=fsb,
                    )

            def vproj_steps():
                nc.sync.dma_start(
                    out=wv_sb, in_=wvs_d[:, :].rearrange("(t p) m -> p t m", p=128)
                )
                nc.sync.dma_start(
                    out=coef_sb, in_=coef_d[0:1, :].rearrange("1 (t p) -> p t", p=128)
                )
                nc.sync.dma_start(out=wo_sb, in_=wos_d[:, :])
                VT_sb = vtmp.tile([128, L], F32)
                xv_b = x_blocks("v", xvT_d)

                def _chunk(ch):
                    sl = slice(ch * 512, (ch + 1) * 512)
                    psp = ps.tile([128, 512], F32, tag="small", bufs=2, name="psp")
                    for t in range(4):
                        nc.tensor.matmul(
                            psp,
                            lhsT=wv_sb[:, t, :],
                            rhs=xv_b[ch][t],
                            start=(t == 0),
                            stop=(t == 3 and not with_bias),
                        )
                    if with_bias:
                        nc.tensor.matmul(
                            psp,
                            lhsT=wb_sb[0:1, 2 * DH2 : 3 * DH2],
                            rhs=ones_sb[0:1, sl],
                            start=False,
                            stop=True,
                        )
                    nc.vector.tensor_copy(out=VT_sb[:, sl], in_=psp)

                def _tp(kt):
                    tp = ps.tile([128, 128], F32, tag="small", bufs=2, name="tp")
                    nc.tensor.transpose(tp, VT_sb[:, kt * 128 : (kt + 1) * 128], ident)
                    for h in range(HPC):
                        nc.vector.tensor_scalar_mul(
                            out=Vp[h][:, kt, 0:DH],
                            in0=tp[:, h * DH : (h + 1) * DH],
                            scalar1=coef_sb[:, kt : kt + 1],
                        )

                for ch in range(4):
                    yield lambda ch=ch: _chunk(ch)
                for kt in range(NKT):
                    yield lambda kt=kt: _tp(kt)

            # ---- emission: software-pipelined over 4 attention units ----
            emit_proj("q", xq_b, wq_sb, 0, qm_sb if with_bias else None, evict_qk(QE))
            emit_proj(
                "k", xk_b, wk_sb, 1, ones_sb if with_bias else None, evict_qk(KE)
            )

            units = [(0, 0), (0, 1), (1, 0), (1, 1)]
            pts = {}
            outps = {}
            nrm2s = {
                0: sbB.tile([DH2, QH], BF16, tag="nrm", bufs=2, name="nrm2_0"),
                1: sbB.tile([DH2, QH], BF16, tag="nrm", bufs=2, name="nrm2_1"),
            }
            # unit 0 scores interleaved with the v projection/transpose
            pts[0] = ptp.tile([128, NKT, QH], ATT_DT, tag="pt", name="pt0")
            interleave(b1_steps(0, 0, pts[0]), vproj_steps())
            for i in range(1, 4):
                qh, h = units[i]
                pqh, ph = units[i - 1]
                pts[i] = ptp.tile([128, NKT, QH], ATT_DT, tag="pt", name=f"pt{i}")
                outps[i - 1] = ps.tile([65, QH], F32, tag="outp", bufs=1, name="outp")
                interleave(
                    b1_steps(qh, h, pts[i]),
                    b2_steps(pqh, ph, pts[i - 1], outps[i - 1]),
                )
                emit_norm(ph, outps[i - 1], nrm2s[pqh])
                if i == 2:
                    for c2 in range(QH // 512):
                        emit_finals_chunk(0, nrm2s[0], c2)
            # ---- tail: unit 3 b2 chunk-major, norm/finals pipelined ----
            # tag "st" reuses a score-PSUM buffer (free once b1 is done), so
            # the tail does not wait for unit 2's norm to release "outp".
            outp3 = ps.tile([65, QH], F32, tag="st", bufs=2, name="outp3")
            for c2 in range(QH // 512):
                for kt in range(NKT):
                    nc.tensor.matmul(
                        outp3[:, c2 * 512 : (c2 + 1) * 512],
                        lhsT=Vp[1][:, kt, :],
                        rhs=pts[3][:, kt, c2 * 512 : (c2 + 1) * 512],
                        start=(kt == 0),
                        stop=(kt == NKT - 1),
                    )
                emit_norm_chunk(1, outp3, nrm2s[1], c2)
            for c2 in range(QH // 512):
                emit_finals_chunk(1, nrm2s[1], c2)

    nc.compile()
    return nc


_CACHE = {}


def _get_nc(with_bias: bool):
    key = ("nc", with_bias)
    if key not in _CACHE:
        _CACHE[key] = build_nc(with_bias)
    return _CACHE[key]


def kernel(q, k, v, text_mask, audio_mask, n_head, wq, bq, wk, bk, wv, bv, wo, bo):
    global LAST_RESULT
    import ml_dtypes

    bf16 = ml_dtypes.bfloat16

    q = np.asarray(q, np.float32)
    k = np.asarray(k, np.float32)
    v = np.asarray(v, np.float32)
    text_mask = np.asarray(text_mask, np.float32)
    audio_mask = np.asarray(audio_mask, np.float32)
    wq = np.asarray(wq, np.float32)
    wk = np.asarray(wk, np.float32)
    wv = np.asarray(wv, np.float32)
    wo = np.asarray(wo, np.float32)
    bq = np.asarray(bq, np.float32)
    bk = np.asarray(bk, np.float32)
    bv = np.asarray(bv, np.float32)
    bo = np.asarray(bo, np.float32)
    assert int(n_head) == H

    with_bias = bool(np.any(bq) or np.any(bk) or np.any(bv))

    pad = np.concatenate([text_mask, audio_mask], axis=1)  # [B, L]
    qm = (pad != 0).astype(np.float32)
    tl = text_mask.sum(1)
    al = audio_mask.sum(1)
    tot = tl + al
    coef = np.concatenate(
        [
            text_mask * (tot / (2.0 * tl))[:, None],
            audio_mask * (tot / (2.0 * al))[:, None],
        ],
        axis=1,
    ).astype(np.float32)
    kbmc = (NEG * (1.0 - qm) - C_LN).astype(np.float32)
    ones_row = np.ones((L,), np.float32)
    cln_row = np.full((L,), C_LN, np.float32)

    def cb(a):
        return np.ascontiguousarray(np.asarray(a, np.float32).astype(bf16))

    def cc(a):
        return np.ascontiguousarray(a, dtype=np.float32)

    in_maps = []
    for core in range(NCORES):
        b, hp = divmod(core, NCORES // B)
        cols = slice(hp * DH2, (hp + 1) * DH2)
        m = {
            "xqT": cb((q[b] * qm[b][:, None]).T),
            "xkT": cb(k[b].T),
            "xvT": cb(v[b].T),
            "wqs": cb(wq.T[:, cols]),
            "wks": cb(wk.T[:, cols] / 8.0),
            "wvs": cb(wv.T[:, cols]),
            "wos": cb(wo.T[cols, :]),
            "aux": cc(np.stack([qm[b], kbmc[b], ones_row, cln_row])),
            "coef": cc(coef[b]).reshape(1, L),
        }
        if with_bias:
            m["wbias"] = cb(
                np.concatenate(
                    [bq[cols], bk[cols] / 8.0, bv[cols], np.zeros(DH2, np.float32)]
                )
            ).reshape(1, 4 * DH2)
        in_maps.append(m)

    res = run_bass_kernel_spmd(
        _get_nc(with_bias), in_maps, core_ids=list(range(NCORES)), trace=TRACE
    )
    LAST_RESULT = res

    out = np.zeros((B, L, DM), np.float32)
    npc = NCORES // B
    for b in range(B):
        acc = res.results[b * npc]["poutT"].astype(np.float32)
        for hp in range(1, npc):
            acc = acc + res.results[b * npc + hp]["poutT"].astype(np.float32)
        out[b] = acc.T + bo[None, :]
    return out


# revision 13
# speedup vs baseline: 1.2664x; 1.0777x over previous
"""Trainium2 Bass kernel for CustomMultiHeadAttention (B=2, L=2048, D=512, H=8).

Sharding: 8 cores = 2 batches x 4 head-pairs. Each core computes, for its
batch b and its 2 heads, the partial output (O_h @ Wo_h summed over its
heads), transposed: poutT [512, 2048]. Host sums the 4 partials per batch,
transposes, and adds bo.

Device-side math per core (all masking folded into matmul contractions):
  Qh = (q[b]*qm) @ WqT_cols + qm*bq_cols          (masked q rows -> exactly 0)
  Kh = k[b] @ WkT_cols/8 + bk_cols/8
  E[k,q] = Kh.Qh + (kb[k]-c)*qm[q] + c            via 2 extra contraction rows
           (kb = -1e4 for masked keys, c = ln(1/2048))
    -> unmasked q: E = s + kb  (masked keys underflow to 0 in exp)
    -> masked q:   E = c       (exp = 1/2048 uniform; denom = 1)
  PT = exp(E)   [k, q] layout
  outT = [Vp | 1]^T @ PT   (Vp = coef * Vh; ones column yields denom row)
  O = outT[0:64] / outT[64]
  poutT[d, q] += Wo[:, d] . O2[:, q]   (both heads packed, contraction 128)

Perf notes vs the first version:
  - x and weights travel as bf16 (halves input DMA), x in [128,512] blocks
    chunk-major so the first projection matmul starts ~2us in.
  - PE warm-up matmuls at t=0 ride the DVFS ramp (0.65->2.4GHz after ~3us
    of continuous busy).
  - Output projection packs both heads into contraction-128 matmuls.
  - reciprocal_approx_fast (~5x faster than reciprocal); norm + finals are
    chunked at 512 cols so the tail pipelines instead of serializing.
  - poutT returns as bf16 (halves output DMA).
"""

import math
import os

os.environ.setdefault("MYCRO_LOCAL_CACHE", "1")

import numpy as np

import concourse.bass as bass
import concourse.tile as tile
from concourse import bacc
from concourse import mybir
from concourse.bass_utils import run_bass_kernel_spmd
from concourse.masks import make_identity

B = 2
L = 2048
DM = 512
H = 8
DH = 64
NCORES = 8
HPC = 2           # heads per core
DH2 = HPC * DH    # 128
NKT = L // 128    # 16 k tiles
QH = 1024         # q chunk for attention phase
NQH = L // QH     # 2
C_LN = -math.log(L)
NEG = -10000.0

F32 = mybir.dt.float32
F32R = mybir.dt.float32r
BF16 = mybir.dt.bfloat16

ATT_DT = BF16     # exp output (PT), V'
QK_DT = F32R      # QE/KE on-chip operands (full-rate, ~tf32 rounding)
X_DT = BF16       # x / w wire + projection matmul dtype

TRACE = False
LAST_RESULT = None

AUX_QM, AUX_KBMC, AUX_ONES, AUX_CLN = 0, 1, 2, 3


def build_nc(with_bias: bool):
    nc = bacc.Bacc(None, target_bir_lowering=False)

    xqT_d = nc.declare_dram_parameter("xqT", [DM, L], X_DT, isOutput=False)
    xkT_d = nc.declare_dram_parameter("xkT", [DM, L], X_DT, isOutput=False)
    xvT_d = nc.declare_dram_parameter("xvT", [DM, L], X_DT, isOutput=False)
    wqs_d = nc.declare_dram_parameter("wqs", [DM, DH2], X_DT, isOutput=False)
    wks_d = nc.declare_dram_parameter("wks", [DM, DH2], X_DT, isOutput=False)
    wvs_d = nc.declare_dram_parameter("wvs", [DM, DH2], X_DT, isOutput=False)
    if with_bias:
        wbias_d = nc.declare_dram_parameter(
            "wbias", [1, 4 * DH2], X_DT, isOutput=False
        )
    wos_d = nc.declare_dram_parameter("wos", [DH2, DM], BF16, isOutput=False)
    aux_d = nc.declare_dram_parameter("aux", [4, L], QK_DT, isOutput=False)
    coef_d = nc.declare_dram_parameter("coef", [1, L], F32, isOutput=False)
    # poutT stored as 16 contiguous [128, 512] blocks, index (qh*2+c2)*4+dt4;
    # host reassembles. Contiguous blocks DMA at full rate (strided rows of
    # the [DM, L] layout only reached ~78 GB/s).
    pout_d = nc.declare_dram_parameter("poutT", [16, 128, 512], BF16, isOutput=True)

    with tile.TileContext(nc) as tc:
        with (
            tc.tile_pool(name="const", bufs=1) as const,
            tc.tile_pool(name="qek", bufs=1) as qek,
            tc.tile_pool(name="xin", bufs=1) as xin,
            tc.tile_pool(name="vtmp", bufs=1) as vtmp,
            tc.tile_pool(name="ptp", bufs=2) as ptp,
            tc.tile_pool(name="sbB", bufs=1) as sbB,
            tc.tile_pool(name="ps", bufs=1, space="PSUM") as ps,
        ):
            # ---- PE warm-up: ride the DVFS ramp while input DMAs land ----
            wrm = const.tile([128, 512], X_DT)
            nc.vector.memset(wrm, 1.0)
            for w in range(7):
                pw = ps.tile([128, 512], F32, tag="small", bufs=2, name=f"warm{w}")
                nc.tensor.matmul(
                    pw, lhsT=wrm[:, 0:128], rhs=wrm, start=True, stop=True
                )

            # ---- constants ----
            ident = const.tile([128, 128], F32)
            make_identity(nc, ident)
            wq_sb = const.tile([128, 4, DH2], X_DT)
            nc.sync.dma_start(
                out=wq_sb, in_=wqs_d[:, :].rearrange("(t p) m -> p t m", p=128)
            )
            wk_sb = const.tile([128, 4, DH2], X_DT)
            nc.sync.dma_start(
                out=wk_sb, in_=wks_d[:, :].rearrange("(t p) m -> p t m", p=128)
            )
            wv_sb = const.tile([128, 4, DH2], X_DT)
            wo_sb = const.tile([DH2, DM], BF16)
            coef_sb = const.tile([128, NKT], F32)
            if with_bias:
                wb_sb = const.tile([1, 4 * DH2], X_DT)
                nc.sync.dma_start(out=wb_sb, in_=wbias_d[:, :])
                qm_sb = const.tile([1, L], X_DT)
                ones_sb = const.tile([1, L], X_DT)
                nc.vector.memset(ones_sb, 1.0)

            # ---- x input blocks: [128, 512], chunk-major so chunk 0 of a
            # projection is computable after 4 small DMAs. Blocks alternate
            # between the two hardware DMA queues (SP + ACT engines) for
            # ~2x aggregate HBM read bandwidth; the ACT engine is idle
            # during the projection phase so its queue is free.
            def x_blocks(pname, xdram, split=True):
                blocks = []
                for ch in range(4):
                    col = []
                    for t in range(4):
                        xt = xin.tile(
                            [128, 512], X_DT, tag="xin", bufs=48,
                            name=f"x{pname}{t}_{ch}",
                        )
                        eng = nc.scalar if (split and t >= 2) else nc.sync
                        eng.dma_start(
                            out=xt,
                            in_=xdram[
                                t * 128 : (t + 1) * 128,
                                ch * 512 : (ch + 1) * 512,
                            ],
                        )
                        col.append(xt)
                    blocks.append(col)
                return blocks

            xq_b = x_blocks("q", xqT_d)
            xk_b = x_blocks("k", xkT_d)
            if with_bias:
                # qm row as X_DT for the bias matmul rhs
                nc.sync.dma_start(out=qm_sb, in_=aux_d[AUX_QM : AUX_QM + 1, :])

            # ---- persistent per-head operands ----
            QE = [qek.tile([66, L], QK_DT, name=f"QE{h}") for h in range(HPC)]
            KE = [qek.tile([66, L], QK_DT, name=f"KE{h}") for h in range(HPC)]
            Vp = [
                qek.tile([128, NKT, DH + 1], ATT_DT, name=f"Vp{h}") for h in range(HPC)
            ]
            # mask/bias rows of the extended operands (DMA direct from host aux)
            for h in range(HPC):
                nc.sync.dma_start(
                    out=QE[h][64:65, :], in_=aux_d[AUX_QM : AUX_QM + 1, :]
                )
                nc.sync.dma_start(
                    out=QE[h][65:66, :], in_=aux_d[AUX_ONES : AUX_ONES + 1, :]
                )
                nc.sync.dma_start(
                    out=KE[h][64:65, :], in_=aux_d[AUX_KBMC : AUX_KBMC + 1, :]
                )
                nc.sync.dma_start(
                    out=KE[h][65:66, :], in_=aux_d[AUX_CLN : AUX_CLN + 1, :]
                )
                nc.vector.memset(Vp[h][:, :, DH : DH + 1], 1.0)

            def emit_proj(pname, xb, w_sb, brow, brhs, evict):
                for ch in range(4):
                    sl = slice(ch * 512, (ch + 1) * 512)
                    psp = ps.tile([128, 512], F32, tag="small", bufs=2, name="psp")
                    for t in range(4):
                        nc.tensor.matmul(
                            psp,
                            lhsT=w_sb[:, t, :],
                            rhs=xb[ch][t],
                            start=(t == 0),
                            stop=(t == 3 and not with_bias),
                        )
                    if with_bias:
                        nc.tensor.matmul(
                            psp,
                            lhsT=wb_sb[0:1, brow * DH2 : (brow + 1) * DH2],
                            rhs=brhs[0:1, sl],
                            start=False,
                            stop=True,
                        )
                    evict(psp, sl)

            def evict_qk(dst):
                def _e(psp, sl):
                    for h in range(HPC):
                        nc.vector.tensor_copy(
                            out=dst[h][0:DH, sl], in_=psp[h * DH : (h + 1) * DH, :]
                        )

                return _e

            def b1_step(qh, h, pt, kt):
                st = ps.tile([128, QH], F32, tag="st", bufs=2, name="st")
                for c2 in range(QH // 512):
                    nc.tensor.matmul(
                        st[:, c2 * 512 : (c2 + 1) * 512],
                        lhsT=KE[h][0:66, kt * 128 : (kt + 1) * 128],
                        rhs=QE[h][
                            0:66, qh * QH + c2 * 512 : qh * QH + (c2 + 1) * 512
                        ],
                        start=True,
                        stop=True,
                    )
                nc.scalar.activation(
                    out=pt[:, kt, :], in_=st, func=mybir.ActivationFunctionType.Exp
                )

            def b1_steps(qh, h, pt):
                for kt in range(NKT):
                    yield lambda kt=kt: b1_step(qh, h, pt, kt)

            def b2_steps(qh, h, pt, outp):
                for kt in range(NKT):
                    def _s(kt=kt):
                        for c2 in range(QH // 512):
                            nc.tensor.matmul(
                                outp[:, c2 * 512 : (c2 + 1) * 512],
                                lhsT=Vp[h][:, kt, :],
                                rhs=pt[:, kt, c2 * 512 : (c2 + 1) * 512],
                                start=(kt == 0),
                                stop=(kt == NKT - 1),
                            )
                    yield _s

            def interleave(*gens):
                gens = [iter(g) for g in gens if g is not None]
                while gens:
                    nxt = []
                    for g in gens:
                        try:
                            next(g)()
                        except StopIteration:
                            continue
                        nxt.append(g)
                    gens = nxt

            from concourse.alu_op_type import AluOpType

            def emit_norm_chunk(h, outp, nrm2, c2):
                sl = slice(c2 * 512, (c2 + 1) * 512)
                den = sbB.tile([1, 512], F32, tag="den", bufs=2, name="den")
                nc.vector.tensor_copy(out=den, in_=outp[DH : DH + 1, sl])
                rcp = sbB.tile([1, 512], F32, tag="rcp", bufs=2, name="rcp")
                nc.vector.reciprocal_approx_fast(out=rcp, in_=den)
                rbc = sbB.tile([DH, 512], F32, tag="rbc", bufs=2, name="rbc")
                nc.gpsimd.partition_broadcast(rbc, rcp[0:1, :], channels=DH)
                # nrm2 = (outp * 1.0) * rbc  — fused PSUM read + scale
                nc.vector.scalar_tensor_tensor(
                    out=nrm2[h * DH : (h + 1) * DH, sl],
                    in0=outp[0:DH, sl],
                    scalar=1.0,
                    in1=rbc,
                    op0=AluOpType.mult,
                    op1=AluOpType.mult,
                )

            def emit_norm(h, outp, nrm2):
                for c2 in range(QH // 512):
                    emit_norm_chunk(h, outp, nrm2, c2)

            def emit_finals_chunk(qh, nrm2, c2, on_scalar=False):
                for dt4 in range(4):
                    fin = ps.tile([128, 512], F32, tag="small", bufs=2, name="fin")
                    nc.tensor.matmul(
                        fin,
                        lhsT=wo_sb[:, dt4 * 128 : (dt4 + 1) * 128],
                        rhs=nrm2[:, c2 * 512 : (c2 + 1) * 512],
                        start=True,
                        stop=True,
                    )
                    fsb = sbB.tile([128, 512], BF16, tag="fsb", bufs=3, name="fsb")
                    if on_scalar:
                        # tail only: the exp stream is done, ACT engine idle
                        nc.scalar.activation(
                            out=fsb, in_=fin,
                            func=mybir.ActivationFunctionType.Copy,
                        )
                    else:
                        nc.vector.tensor_copy(out=fsb, in_=fin)
                    # contiguous pout block: (qh, c2, dt4) -> [128, 512]
                    blk = (qh * 2 + c2) * 4 + dt4
                    nc.sync.dma_start(out=pout_d[blk], in_=fsb)

            def vproj_steps():
                nc.sync.dma_start(
                    out=wv_sb, in_=wvs_d[:, :].rearrange("(t p) m -> p t m", p=128)
                )
                nc.sync.dma_start(
                    out=coef_sb, in_=coef_d[0:1, :].rearrange("1 (t p) -> p t", p=128)
                )
                nc.sync.dma_start(out=wo_sb, in_=wos_d[:, :])
                VT_sb = vtmp.tile([128, L], F32)
                # all on the SP queue: the ACT engine is mid-exp-stream here
                xv_b = x_blocks("v", xvT_d, split=False)

                def _chunk(ch):
                    sl = slice(ch * 512, (ch + 1) * 512)
                    psp = ps.tile([128, 512], F32, tag="small", bufs=2, name="psp")
                    for t in range(4):
                        nc.tensor.matmul(
                            psp,
                            lhsT=wv_sb[:, t, :],
                            rhs=xv_b[ch][t],
                            start=(t == 0),
                            stop=(t == 3 and not with_bias),
                        )
                    if with_bias:
                        nc.tensor.matmul(
                            psp,
                            lhsT=wb_sb[0:1, 2 * DH2 : 3 * DH2],
                            rhs=ones_sb[0:1, sl],
                            start=False,
                            stop=True,
                        )
                    nc.vector.tensor_copy(out=VT_sb[:, sl], in_=psp)

                def _tp(kt):
                    tp = ps.tile([128, 128], F32, tag="small", bufs=2, name="tp")
                    nc.tensor.transpose(tp, VT_sb[:, kt * 128 : (kt + 1) * 128], ident)
                    for h in range(HPC):
                        nc.vector.tensor_scalar_mul(
                            out=Vp[h][:, kt, 0:DH],
                            in0=tp[:, h * DH : (h + 1) * DH],
                            scalar1=coef_sb[:, kt : kt + 1],
                        )

                for ch in range(4):
                    yield lambda ch=ch: _chunk(ch)
                for kt in range(NKT):
                    yield lambda kt=kt: _tp(kt)

            # ---- emission: software-pipelined over 4 attention units ----
            emit_proj("q", xq_b, wq_sb, 0, qm_sb if with_bias else None, evict_qk(QE))
            emit_proj(
                "k", xk_b, wk_sb, 1, ones_sb if with_bias else None, evict_qk(KE)
            )

            units = [(0, 0), (0, 1), (1, 0), (1, 1)]
            pts = {}
            outps = {}
            nrm2s = {
                0: sbB.tile([DH2, QH], BF16, tag="nrm", bufs=2, name="nrm2_0"),
                1: sbB.tile([DH2, QH], BF16, tag="nrm", bufs=2, name="nrm2_1"),
            }
            # unit 0 scores interleaved with the v projection/transpose
            pts[0] = ptp.tile([128, NKT, QH], ATT_DT, tag="pt", name="pt0")
            interleave(b1_steps(0, 0, pts[0]), vproj_steps())
            for i in range(1, 4):
                qh, h = units[i]
                pqh, ph = units[i - 1]
                pts[i] = ptp.tile([128, NKT, QH], ATT_DT, tag="pt", name=f"pt{i}")
                outps[i - 1] = ps.tile([65, QH], F32, tag="outp", bufs=1, name="outp")
                interleave(
                    b1_steps(qh, h, pts[i]),
                    b2_steps(pqh, ph, pts[i - 1], outps[i - 1]),
                )
                emit_norm(ph, outps[i - 1], nrm2s[pqh])
                if i == 2:
                    for c2 in range(QH // 512):
                        emit_finals_chunk(0, nrm2s[0], c2)
            # ---- tail: unit 3 b2 chunk-major, norm/finals pipelined ----
            # tag "st" reuses a score-PSUM buffer (free once b1 is done), so
            # the tail does not wait for unit 2's norm to release "outp".
            outp3 = ps.tile([65, QH], F32, tag="st", bufs=2, name="outp3")
            for c2 in range(QH // 512):
                for kt in range(NKT):
                    nc.tensor.matmul(
                        outp3[:, c2 * 512 : (c2 + 1) * 512],
                        lhsT=Vp[1][:, kt, :],
                        rhs=pts[3][:, kt, c2 * 512 : (c2 + 1) * 512],
                        start=(kt == 0),
                        stop=(kt == NKT - 1),
                    )
                emit_norm_chunk(1, outp3, nrm2s[1], c2)
            for c2 in range(QH // 512):
                emit_finals_chunk(1, nrm2s[1], c2, on_scalar=True)

    nc.compile()
    return nc


_CACHE = {}


def _get_nc(with_bias: bool):
    key = ("nc", with_bias)
    if key not in _CACHE:
        _CACHE[key] = build_nc(with_bias)
    return _CACHE[key]


def kernel(q, k, v, text_mask, audio_mask, n_head, wq, bq, wk, bk, wv, bv, wo, bo):
    global LAST_RESULT
    import ml_dtypes

    bf16 = ml_dtypes.bfloat16

    q = np.asarray(q, np.float32)
    k = np.asarray(k, np.float32)
    v = np.asarray(v, np.float32)
    text_mask = np.asarray(text_mask, np.float32)
    audio_mask = np.asarray(audio_mask, np.float32)
    wq = np.asarray(wq, np.float32)
    wk = np.asarray(wk, np.float32)
    wv = np.asarray(wv, np.float32)
    wo = np.asarray(wo, np.float32)
    bq = np.asarray(bq, np.float32)
    bk = np.asarray(bk, np.float32)
    bv = np.asarray(bv, np.float32)
    bo = np.asarray(bo, np.float32)
    assert int(n_head) == H

    with_bias = bool(np.any(bq) or np.any(bk) or np.any(bv))

    pad = np.concatenate([text_mask, audio_mask], axis=1)  # [B, L]
    qm = (pad != 0).astype(np.float32)
    tl = text_mask.sum(1)
    al = audio_mask.sum(1)
    tot = tl + al
    coef = np.concatenate(
        [
            text_mask * (tot / (2.0 * tl))[:, None],
            audio_mask * (tot / (2.0 * al))[:, None],
        ],
        axis=1,
    ).astype(np.float32)
    kbmc = (NEG * (1.0 - qm) - C_LN).astype(np.float32)
    ones_row = np.ones((L,), np.float32)
    cln_row = np.full((L,), C_LN, np.float32)

    def cb(a):
        return np.ascontiguousarray(np.asarray(a, np.float32).astype(bf16))

    def cc(a):
        return np.ascontiguousarray(a, dtype=np.float32)

    in_maps = []
    for core in range(NCORES):
        b, hp = divmod(core, NCORES // B)
        cols = slice(hp * DH2, (hp + 1) * DH2)
        m = {
            "xqT": cb((q[b] * qm[b][:, None]).T),
            "xkT": cb(k[b].T),
            "xvT": cb(v[b].T),
            "wqs": cb(wq.T[:, cols]),
            "wks": cb(wk.T[:, cols] / 8.0),
            "wvs": cb(wv.T[:, cols]),
            "wos": cb(wo.T[cols, :]),
            "aux": cc(np.stack([qm[b], kbmc[b], ones_row, cln_row])),
            "coef": cc(coef[b]).reshape(1, L),
        }
        if with_bias:
            m["wbias"] = cb(
                np.concatenate(
                    [bq[cols], bk[cols] / 8.0, bv[cols], np.zeros(DH2, np.float32)]
                )
            ).reshape(1, 4 * DH2)
        in_maps.append(m)

    res = run_bass_kernel_spmd(
        _get_nc(with_bias), in_maps, core_ids=list(range(NCORES)), trace=TRACE
    )
    LAST_RESULT = res

    def unblock(arr):
        # [16,128,512] blocks (qh,c2,dt4) -> poutT [DM, L]
        return (
            arr.reshape(2, 2, 4, 128, 512)
            .transpose(2, 3, 0, 1, 4)
            .reshape(DM, L)
        )

    out = np.zeros((B, L, DM), np.float32)
    npc = NCORES // B
    for b in range(B):
        acc = res.results[b * npc]["poutT"].astype(np.float32)
        for hp in range(1, npc):
            acc = acc + res.results[b * npc + hp]["poutT"].astype(np.float32)
        out[b] = unblock(acc).T + bo[None, :]
    return out
